# revision 4
# baseline (speedup 1.0000x reference)
"""AhaDiffuser Trainium2 kernel.

Key algebraic fact: the reference returns b[:, -1, :] and every op is
pointwise in t, so the output depends only on h[:, -1, :] ([B, D]) and
targets[:, -1] ([B]).  The remaining heavy work is streaming the facet
(K x D x V) and state (K x D x D) weights through the TensorEngine once,
which is HBM-bandwidth bound.

Sharding (expert-parallel, per the hint): core k owns facet_w[k]/facet_b[k]
and state_w[k].  Each core computes, for its expert:
  z      = h_last @ facet_w[k]            [B, V]   (bf16 weights; decisions
                                                    have |s-0.7| margin ~0.25,
                                                    bf16 error ~0.003)
  sumexp = sum_v exp(z)  (per B, fused exp+accum on ScalarE)
  z_t    = z[b, targets[b, -1]]           (static offsets baked at build)
  states = h_last @ state_w[k]            [B, D]   (f32)
Host gathers the tiny [B] partials + [B, D] states, then does the [B, K]
gate/boost logic, combine, layernorm and compress in float64.
"""

import numpy as np
import ml_dtypes

B, T, D, K, V = 2, 1024, 1024, 8, 8192
NKC = D // 128            # contraction chunks of 128
NVB = 8                   # facet V blocks per core
VB = V // NVB             # 1024 columns per block
NSB = 2                   # state output-D blocks (512 each)
S_THRESH, BOOST_GAIN, MAX_PAIRS, EPS = 0.7, 2.0, 1, 1e-9

FACET_DT_NAME = "bfloat16"          # mybir dtype name for facet weights & h
_FACET_NP = {"bfloat16": ml_dtypes.bfloat16,
             "float8e4": ml_dtypes.float8_e4m3}[FACET_DT_NAME]
H_FACET_NP = ml_dtypes.bfloat16     # stationary h dtype for the facet matmul

_PROGRAM_CACHE = {}


def _build_program(t_idx, add_facet_bias):
    import concourse.bacc as bacc
    import concourse.tile as tile
    import concourse.mybir as mybir

    dt = mybir.dt
    fdt = getattr(dt, FACET_DT_NAME)
    hdt = dt.bfloat16

    nc = bacc.Bacc("TRN2", target_bir_lowering=False, debug=False)

    hTb = nc.dram_tensor("hTb", [128, NKC * B], hdt, kind="ExternalInput").ap()
    hTf = nc.dram_tensor("hTf", [128, NKC * B], dt.float32, kind="ExternalInput").ap()
    fw = nc.dram_tensor("fw", [128, NVB, NKC * VB], fdt, kind="ExternalInput").ap()
    sw = nc.dram_tensor("sw", [128, NSB, NKC * 512], dt.float32, kind="ExternalInput").ap()
    if add_facet_bias:
        fbb = nc.dram_tensor("fbb", [B, NVB, VB], dt.float32, kind="ExternalInput").ap()
    # osc: cols 0..NVB-1 = per-block sum(exp(z));
    # col NVB+b = z[:, t_idx[b]] (host reads row b) -- ACT ops can't start
    # at partition 1, so each target column is copied for both partitions.
    osc = nc.dram_tensor("osc", [B, NVB + 2], dt.float32, kind="ExternalOutput").ap()
    ost = nc.dram_tensor("ost", [B, D], dt.float32, kind="ExternalOutput").ap()

    with tile.TileContext(nc) as tc:
        with (
            tc.tile_pool(name="const", bufs=1) as const,
            tc.tile_pool(name="fwp", bufs=NVB) as fwp,       # fully resident
            tc.tile_pool(name="swp", bufs=NSB) as swp,
            tc.tile_pool(name="scratch", bufs=2) as scratch,
            tc.tile_pool(name="fbp", bufs=2) as fbp,
            tc.tile_pool(name="psf", bufs=3, space="PSUM") as psf,
            tc.tile_pool(name="pss", bufs=2, space="PSUM") as pss,
        ):
            hb = const.tile([128, NKC * B], hdt)
            nc.sync.dma_start(hb[:], hTb)
            hf = const.tile([128, NKC * B], dt.float32)
            nc.sync.dma_start(hf[:], hTf)

            osc_sb = const.tile([B, NVB + 2], dt.float32)
            ost_sb = const.tile([B, D], dt.float32)

            # ---- states first: their f32 (2-pass) matmuls and the sw DMA are
            # the slow pieces; run them under the facet weight stream so the
            # kernel tail is the cheap facet path of the last block.
            for sb_i in range(NSB):
                swt = swp.tile([128, NKC * 512], dt.float32)
                nc.sync.dma_start(swt[:], sw[:, sb_i, :])
                ss = pss.tile([B, 512], dt.float32)
                for kc in range(NKC):
                    nc.tensor.matmul(
                        ss[:],
                        hf[:, kc * B:(kc + 1) * B],
                        swt[:, kc * 512:(kc + 1) * 512],
                        start=(kc == 0),
                        stop=(kc == NKC - 1),
                    )
                nc.scalar.copy(ost_sb[:, sb_i * 512:(sb_i + 1) * 512], ss[:])
            nc.sync.dma_start(ost, ost_sb[:])

            # ---- facet: z = h @ fw[k], fused exp+sum, static target pick ----
            for nb in range(NVB):
                fwt = fwp.tile([128, NKC * VB], fdt)
                nc.sync.dma_start(fwt[:], fw[:, nb, :])
                ps = psf.tile([B, VB], dt.float32)
                for half in range(VB // 512):
                    for kc in range(NKC):
                        nc.tensor.matmul(
                            ps[:, half * 512:(half + 1) * 512],
                            hb[:, kc * B:(kc + 1) * B],
                            fwt[:, kc * VB + half * 512: kc * VB + half * 512 + 512],
                            start=(kc == 0),
                            stop=(kc == NKC - 1),
                        )
                if add_facet_bias:
                    fbt = fbp.tile([B, VB], dt.float32)
                    nc.sync.dma_start(fbt[:], fbb[:, nb, :])
                    nc.vector.tensor_add(ps[:], ps[:], fbt[:])
                ex = scratch.tile([B, VB], dt.float32)
                nc.scalar.activation(
                    ex[:],
                    ps[:],
                    mybir.ActivationFunctionType.Exp,
                    accum_out=osc_sb[:, nb: nb + 1],
                )
                for b in range(B):
                    if t_idx[b] // VB == nb:
                        off = t_idx[b] % VB
                        nc.scalar.copy(
                            osc_sb[:, NVB + b: NVB + b + 1],
                            ps[:, off: off + 1],
                        )

            nc.sync.dma_start(osc, osc_sb[:])

    nc.compile()
    return nc


def _prep_core_inputs(h_last, facet_w, facet_b, state_w, add_facet_bias):
    """Per-core input dicts (expert-parallel: core k owns expert k)."""
    hT = np.ascontiguousarray(h_last.T.astype(np.float32))          # [D, B]
    hpre = hT.reshape(NKC, 128, B).transpose(1, 0, 2).reshape(128, NKC * B)
    hTf = np.ascontiguousarray(hpre)
    hTb = np.ascontiguousarray(hpre.astype(H_FACET_NP))

    in_maps = []
    for k in range(K):
        A = facet_w[k]                                              # [D, V] f32
        fw_pre = np.ascontiguousarray(
            A.reshape(NKC, 128, NVB, VB).transpose(1, 2, 0, 3)
        ).astype(_FACET_NP).reshape(128, NVB, NKC * VB)
        S = state_w[k].astype(np.float32)                           # [D, D]
        sw_pre = np.ascontiguousarray(
            S.reshape(NKC, 128, NSB, 512).transpose(1, 2, 0, 3)
        ).reshape(128, NSB, NKC * 512)
        m = {"hTb": hTb, "hTf": hTf, "fw": fw_pre, "sw": sw_pre}
        if add_facet_bias:
            m["fbb"] = np.ascontiguousarray(
                np.broadcast_to(facet_b[k].astype(np.float32), (B, V))
            ).reshape(B, NVB, VB)
        in_maps.append(m)
    return in_maps


def _run_device(t_idx, add_facet_bias, in_maps, trace=False):
    from concourse.bass_utils import run_bass_kernel_spmd

    key = (tuple(t_idx), bool(add_facet_bias))
    nc = _PROGRAM_CACHE.get(key)
    if nc is None:
        nc = _build_program(t_idx, add_facet_bias)
        _PROGRAM_CACHE[key] = nc
    res = run_bass_kernel_spmd(
        nc, in_maps, list(range(K)),
        trace=trace, trace_cores=list(range(K)) if trace else None,
    )
    return res


def kernel(h, targets, em_gate_w, em_gate_b, state_w, state_b,
           mfs_gate_w, mfs_gate_b, facet_w, facet_b,
           ln_scale, ln_bias, compress_w, compress_b,
           _trace=False, _result_box=None):
    h = np.asarray(h)
    targets = np.asarray(targets)

    h_last = h[:, -1, :].astype(np.float64)                          # [B, D]
    t_idx = [int(targets[b, -1]) for b in range(B)]
    add_facet_bias = bool(np.any(np.asarray(facet_b)))

    in_maps = _prep_core_inputs(h_last, np.asarray(facet_w, np.float32),
                                np.asarray(facet_b, np.float32),
                                np.asarray(state_w), add_facet_bias)
    res = _run_device(t_idx, add_facet_bias, in_maps, trace=_trace)
    if _result_box is not None:
        _result_box.append(res)

    # ---- host combine (tiny: [B, K] logic + LN + compress) ----
    sumexp = np.zeros((B, K))
    z_t = np.zeros((B, K))
    states = np.zeros((B, K, D))
    for k in range(K):
        osc = res.results[k]["osc"].astype(np.float64)               # [B, NVB+2]
        sumexp[:, k] = osc[:, :NVB].sum(-1)
        z_t[:, k] = osc[np.arange(B), NVB + np.arange(B)]
        states[:, k, :] = res.results[k]["ost"].astype(np.float64)
    states += np.asarray(state_b, np.float64)[None, :, :]
    logp = z_t - np.log(sumexp)                                      # [B, K]

    def softmax64(x):
        e = np.exp(x - x.max(-1, keepdims=True))
        return e / e.sum(-1, keepdims=True)

    G = softmax64(h_last @ np.asarray(em_gate_w, np.float64)
                  + np.asarray(em_gate_b, np.float64))
    g = softmax64(h_last @ np.asarray(mfs_gate_w, np.float64)
                  + np.asarray(mfs_gate_b, np.float64))

    seli2 = np.argsort(-G, axis=-1, kind="stable")[:, :2]            # top-2, ties->low idx
    sel_mask = np.zeros((B, K), bool)
    for b in range(B):
        sel_mask[b, seli2[b]] = True

    logg = np.log(np.maximum(g, 1e-9))
    mix = logg + logp
    mmax = mix.max(-1, keepdims=True)
    log_mix = mmax[..., 0] + np.log(np.exp(mix - mmax).sum(-1))
    s = logp - log_mix[..., None]

    aha = (s > S_THRESH) & (~sel_mask)
    boosted = G * np.where(aha, BOOST_GAIN, 1.0)
    sel_add = np.zeros((B, K))
    for b in range(B):
        sel_add[b, seli2[b, 0]] = 0.5
    boosted = np.where(aha.any(-1, keepdims=True), boosted + sel_add, boosted)
    boosted = boosted / np.maximum(boosted.sum(-1, keepdims=True), EPS)

    bvec = np.einsum("bk,bkd->bd", boosted, states)
    mu = bvec.mean(-1, keepdims=True)
    var = ((bvec - mu) ** 2).mean(-1, keepdims=True)
    ln = (bvec - mu) / np.sqrt(var + 1e-5) * np.asarray(ln_scale, np.float64) \
         + np.asarray(ln_bias, np.float64)
    out = ln @ np.asarray(compress_w, np.float64) + np.asarray(compress_b, np.float64)
    return out.astype(np.float32)


# revision 7
# speedup vs baseline: 1.0745x; 1.0745x over previous
"""AhaDiffuser Trainium2 kernel.

Key algebraic fact: the reference returns b[:, -1, :] and every op is
pointwise in t, so the output depends only on h[:, -1, :] ([B, D]) and
targets[:, -1] ([B]).  The remaining heavy work is streaming the facet
(K x D x V) and state (K x D x D) weights through the TensorEngine once,
which is HBM-bandwidth bound.

Sharding (expert-parallel, per the hint): core k owns facet_w[k]/facet_b[k]
and state_w[k].  Each core computes, for its expert:
  z      = h_last @ facet_w[k]            [B, V]   (bf16 weights; decisions
                                                    have |s-0.7| margin ~0.25,
                                                    bf16 error ~0.003)
  sumexp = sum_v exp(z)  (per B, fused exp+accum on ScalarE)
  z_t    = z[b, targets[b, -1]]           (static offsets baked at build)
  states = h_last @ state_w[k]            [B, D]   (f32)
Host gathers the tiny [B] partials + [B, D] states, then does the [B, K]
gate/boost logic, combine, layernorm and compress in float64.
"""

import numpy as np
import ml_dtypes

B, T, D, K, V = 2, 1024, 1024, 8, 8192
NKC = D // 128            # contraction chunks of 128
NVB = 8                   # facet V blocks per core
VB = V // NVB             # 1024 columns per block
NSB = 2                   # state output-D blocks (512 each)
NPAIR = 4                 # facet DMA pair-blocks (2 v-blocks each)
NCH = 16                  # facet v-chunks of 512 (4 per pair, col-tiled)
S_THRESH, BOOST_GAIN, MAX_PAIRS, EPS = 0.7, 2.0, 1, 1e-9

FACET_DT_NAME = "bfloat16"          # mybir dtype name for facet weights & h
_FACET_NP = {"bfloat16": ml_dtypes.bfloat16,
             "float8e4": ml_dtypes.float8_e4m3}[FACET_DT_NAME]
H_FACET_NP = ml_dtypes.bfloat16     # stationary h dtype for the facet matmul

_PROGRAM_CACHE = {}


def _build_program(t_idx, add_facet_bias):
    import concourse.bacc as bacc
    import concourse.tile as tile
    import concourse.mybir as mybir

    dt = mybir.dt
    fdt = getattr(dt, FACET_DT_NAME)
    hdt = dt.bfloat16

    nc = bacc.Bacc("TRN2", target_bir_lowering=False, debug=False)

    hTb = nc.dram_tensor("hTb", [128, NKC * B], hdt, kind="ExternalInput").ap()
    hTf = nc.dram_tensor("hTf", [128, NKC * B], dt.float32, kind="ExternalInput").ap()
    # facet weights in 4 pair-blocks of 2 MiB; pair p holds v-chunks 4p..4p+3,
    # free layout (kc, cc, 512)
    fw = nc.dram_tensor("fw", [128, NPAIR, NKC * 4 * 512], fdt, kind="ExternalInput").ap()
    sw = nc.dram_tensor("sw", [128, NSB, NKC * 512], dt.float32, kind="ExternalInput").ap()
    if add_facet_bias:
        fbb = nc.dram_tensor("fbb", [128, NPAIR, 512], dt.float32, kind="ExternalInput").ap()
    # osc rows 32*(c%4)+b: cols 0..NCH-1 = per-chunk sum(exp(z)), col NCH+b = z_target
    osc = nc.dram_tensor("osc", [128, NCH + 2], dt.float32, kind="ExternalOutput").ap()
    ost = nc.dram_tensor("ost", [B, D], dt.float32, kind="ExternalOutput").ap()

    with tile.TileContext(nc) as tc:
        with (
            tc.tile_pool(name="const", bufs=1) as const,
            tc.tile_pool(name="fwp", bufs=NPAIR) as fwp,     # fully resident
            tc.tile_pool(name="swp", bufs=NSB) as swp,
            tc.tile_pool(name="scratch", bufs=2) as scratch,
            tc.tile_pool(name="fbp", bufs=2) as fbp,
            tc.tile_pool(name="psf", bufs=3, space="PSUM") as psf,
            tc.tile_pool(name="pss", bufs=2, space="PSUM") as pss,
        ):
            # tiny h loads ride the SWDGE (gpsimd) rings so the sync HWDGE
            # FIFO starts streaming weights immediately
            hb = const.tile([128, NKC * B], hdt)
            nc.gpsimd.dma_start(hb[:], hTb)
            hf = const.tile([128, NKC * B], dt.float32)
            nc.gpsimd.dma_start(hf[:], hTf)

            osc_sb = const.tile([128, NCH + 2], dt.float32)
            nc.gpsimd.memset(osc_sb[:], 0.0)
            ost_sb = const.tile([B, D], dt.float32)

            # ---- states first: the f32 (2-pass) matmuls and sw DMAs run
            # under the facet weight stream, keeping the kernel tail cheap.
            for sb_i in range(NSB):
                swt = swp.tile([128, NKC * 512], dt.float32)
                nc.sync.dma_start(swt[:], sw[:, sb_i, :])
                ss = pss.tile([B, 512], dt.float32)
                for kc in range(NKC):
                    nc.tensor.matmul(
                        ss[:],
                        hf[:, kc * B:(kc + 1) * B],
                        swt[:, kc * 512:(kc + 1) * 512],
                        start=(kc == 0),
                        stop=(kc == NKC - 1),
                    )
                nc.scalar.copy(ost_sb[:, sb_i * 512:(sb_i + 1) * 512], ss[:])
            nc.scalar.dma_start(ost, ost_sb[:])

            # ---- facet: 4 v-chunks packed per PSUM bank via PE column
            # tiling; 4 matmuls stream concurrently through distinct column
            # groups of the array.
            for t in range(NPAIR):
                fwt = fwp.tile([128, NKC * 4 * 512], fdt)
                nc.sync.dma_start(fwt[:], fw[:, t, :])
                pt = psf.tile([128, 512], dt.float32)
                for kc in range(NKC):
                    for cc in range(4):
                        nc.tensor.matmul(
                            pt[32 * cc:32 * cc + B, :],
                            hb[:, kc * B:(kc + 1) * B],
                            fwt[:, kc * 2048 + cc * 512: kc * 2048 + cc * 512 + 512],
                            start=(kc == 0),
                            stop=(kc == NKC - 1),
                            tile_position=(0, 32 * cc),
                        )
                if add_facet_bias:
                    fbt = fbp.tile([128, 512], dt.float32)
                    nc.sync.dma_start(fbt[:], fbb[:, t, :])
                    nc.vector.tensor_add(pt[:], pt[:], fbt[:])
                ex = scratch.tile([128, 512], dt.float32)
                for cc in range(4):
                    c = t * 4 + cc
                    nc.scalar.activation(
                        ex[32 * cc:32 * cc + B, :],
                        pt[32 * cc:32 * cc + B, :],
                        mybir.ActivationFunctionType.Exp,
                        accum_out=osc_sb[32 * cc:32 * cc + B, c: c + 1],
                    )
                for b in range(B):
                    if t_idx[b] // 2048 == t:
                        j = (t_idx[b] // 512) % 4
                        off = t_idx[b] % 512
                        nc.scalar.copy(
                            osc_sb[32 * j:32 * j + B, NCH + b: NCH + b + 1],
                            pt[32 * j:32 * j + B, off: off + 1],
                        )

            nc.scalar.dma_start(osc, osc_sb[:])

    nc.compile()
    return nc


def _prep_core_inputs(h_last, facet_w, facet_b, state_w, add_facet_bias):
    """Per-core input dicts (expert-parallel: core k owns expert k)."""
    hT = np.ascontiguousarray(h_last.T.astype(np.float32))          # [D, B]
    hpre = hT.reshape(NKC, 128, B).transpose(1, 0, 2).reshape(128, NKC * B)
    hTf = np.ascontiguousarray(hpre)
    hTb = np.ascontiguousarray(hpre.astype(H_FACET_NP))

    in_maps = []
    for k in range(K):
        A = facet_w[k]                                              # [D, V] f32
        fw_pre = np.ascontiguousarray(
            A.reshape(NKC, 128, NPAIR, 4, 512).transpose(1, 2, 0, 3, 4)
        ).astype(_FACET_NP).reshape(128, NPAIR, NKC * 4 * 512)
        S = state_w[k].astype(np.float32)                           # [D, D]
        sw_pre = np.ascontiguousarray(
            S.reshape(NKC, 128, NSB, 512).transpose(1, 2, 0, 3)
        ).reshape(128, NSB, NKC * 512)
        m = {"hTb": hTb, "hTf": hTf, "fw": fw_pre, "sw": sw_pre}
        if add_facet_bias:
            fbb = np.zeros((128, NPAIR, 512), np.float32)
            fb = facet_b[k].astype(np.float32)
            for c in range(NCH):
                pair, j = c // 4, c % 4
                fbb[32 * j:32 * j + B, pair, :] = fb[c * 512:(c + 1) * 512]
            m["fbb"] = fbb
        in_maps.append(m)
    return in_maps


def _run_device(t_idx, add_facet_bias, in_maps, trace=False):
    from concourse.bass_utils import run_bass_kernel_spmd

    key = (tuple(t_idx), bool(add_facet_bias))
    nc = _PROGRAM_CACHE.get(key)
    if nc is None:
        nc = _build_program(t_idx, add_facet_bias)
        _PROGRAM_CACHE[key] = nc
    res = run_bass_kernel_spmd(
        nc, in_maps, list(range(K)),
        trace=trace, trace_cores=list(range(K)) if trace else None,
    )
    return res


def kernel(h, targets, em_gate_w, em_gate_b, state_w, state_b,
           mfs_gate_w, mfs_gate_b, facet_w, facet_b,
           ln_scale, ln_bias, compress_w, compress_b,
           _trace=False, _result_box=None):
    h = np.asarray(h)
    targets = np.asarray(targets)

    h_last = h[:, -1, :].astype(np.float64)                          # [B, D]
    t_idx = [int(targets[b, -1]) for b in range(B)]
    add_facet_bias = bool(np.any(np.asarray(facet_b)))

    in_maps = _prep_core_inputs(h_last, np.asarray(facet_w, np.float32),
                                np.asarray(facet_b, np.float32),
                                np.asarray(state_w), add_facet_bias)
    res = _run_device(t_idx, add_facet_bias, in_maps, trace=_trace)
    if _result_box is not None:
        _result_box.append(res)

    # ---- host combine (tiny: [B, K] logic + LN + compress) ----
    sumexp = np.zeros((B, K))
    z_t = np.zeros((B, K))
    states = np.zeros((B, K, D))
    for k in range(K):
        osc = res.results[k]["osc"].astype(np.float64)               # [128, NCH+2]
        for c in range(NCH):
            j = c % 4
            for b in range(B):
                sumexp[b, k] += osc[32 * j + b, c]
        for b in range(B):
            jb = (t_idx[b] // 512) % 4
            z_t[b, k] = osc[32 * jb + b, NCH + b]
        states[:, k, :] = res.results[k]["ost"].astype(np.float64)
    states += np.asarray(state_b, np.float64)[None, :, :]
    logp = z_t - np.log(sumexp)                                      # [B, K]

    def softmax64(x):
        e = np.exp(x - x.max(-1, keepdims=True))
        return e / e.sum(-1, keepdims=True)

    G = softmax64(h_last @ np.asarray(em_gate_w, np.float64)
                  + np.asarray(em_gate_b, np.float64))
    g = softmax64(h_last @ np.asarray(mfs_gate_w, np.float64)
                  + np.asarray(mfs_gate_b, np.float64))

    seli2 = np.argsort(-G, axis=-1, kind="stable")[:, :2]            # top-2, ties->low idx
    sel_mask = np.zeros((B, K), bool)
    for b in range(B):
        sel_mask[b, seli2[b]] = True

    logg = np.log(np.maximum(g, 1e-9))
    mix = logg + logp
    mmax = mix.max(-1, keepdims=True)
    log_mix = mmax[..., 0] + np.log(np.exp(mix - mmax).sum(-1))
    s = logp - log_mix[..., None]

    aha = (s > S_THRESH) & (~sel_mask)
    boosted = G * np.where(aha, BOOST_GAIN, 1.0)
    sel_add = np.zeros((B, K))
    for b in range(B):
        sel_add[b, seli2[b, 0]] = 0.5
    boosted = np.where(aha.any(-1, keepdims=True), boosted + sel_add, boosted)
    boosted = boosted / np.maximum(boosted.sum(-1, keepdims=True), EPS)

    bvec = np.einsum("bk,bkd->bd", boosted, states)
    mu = bvec.mean(-1, keepdims=True)
    var = ((bvec - mu) ** 2).mean(-1, keepdims=True)
    ln = (bvec - mu) / np.sqrt(var + 1e-5) * np.asarray(ln_scale, np.float64) \
         + np.asarray(ln_bias, np.float64)
    out = ln @ np.asarray(compress_w, np.float64) + np.asarray(compress_b, np.float64)
    return out.astype(np.float32)


# revision 8
# speedup vs baseline: 1.5291x; 1.4230x over previous
"""AhaDiffuser Trainium2 kernel.

Key algebraic fact: the reference returns b[:, -1, :] and every op is
pointwise in t, so the output depends only on h[:, -1, :] ([B, D]) and
targets[:, -1] ([B]).  The remaining heavy work is streaming the facet
(K x D x V) and state (K x D x D) weights through the TensorEngine once,
which is HBM-bandwidth bound.

Sharding (expert-parallel, per the hint): core k owns facet_w[k]/facet_b[k]
and state_w[k].  Each core computes, for its expert:
  z      = h_last @ facet_w[k]            [B, V]   (bf16 weights; decisions
                                                    have |s-0.7| margin ~0.25,
                                                    bf16 error ~0.003)
  sumexp = sum_v exp(z)  (per B, fused exp+accum on ScalarE)
  z_t    = z[b, targets[b, -1]]           (static offsets baked at build)
  states = h_last @ state_w[k]            [B, D]   (f32)
Host gathers the tiny [B] partials + [B, D] states, then does the [B, K]
gate/boost logic, combine, layernorm and compress in float64.
"""

import numpy as np
import ml_dtypes

B, T, D, K, V = 2, 1024, 1024, 8, 8192
NKC = D // 128            # contraction chunks of 128
NVB = 8                   # facet V blocks per core
VB = V // NVB             # 1024 columns per block
NSB = 2                   # state output-D blocks (512 each)
NPAIR = 4                 # facet DMA pair-blocks (2 v-blocks each)
NCH = 16                  # facet v-chunks of 512 (4 per pair, col-tiled)
S_THRESH, BOOST_GAIN, MAX_PAIRS, EPS = 0.7, 2.0, 1, 1e-9

FACET_DT_NAME = "float8e4"          # facet weights dtype; decisions have
                                    # |s-0.7| margin ~0.25 vs ~0.05 fp8 error
_FACET_NP = {"bfloat16": ml_dtypes.bfloat16,
             "float8e4": ml_dtypes.float8_e4m3}[FACET_DT_NAME]
H_FACET_NP = ml_dtypes.bfloat16     # stationary h dtype for the facet matmul

_PROGRAM_CACHE = {}


def _build_program(t_idx, add_facet_bias):
    import concourse.bacc as bacc
    import concourse.tile as tile
    import concourse.mybir as mybir

    dt = mybir.dt
    fdt = getattr(dt, FACET_DT_NAME)
    hdt = dt.bfloat16

    nc = bacc.Bacc("TRN2", target_bir_lowering=False, debug=False)

    hTb = nc.dram_tensor("hTb", [128, NKC * B], hdt, kind="ExternalInput").ap()
    hTf = nc.dram_tensor("hTf", [128, NKC * B], dt.float32, kind="ExternalInput").ap()
    # facet weights in 4 pair-blocks of 2 MiB; pair p holds v-chunks 4p..4p+3,
    # free layout (kc, cc, 512)
    fw = nc.dram_tensor("fw", [128, NPAIR, NKC * 4 * 512], fdt, kind="ExternalInput").ap()
    sw = nc.dram_tensor("sw", [128, NSB, NKC * 512], dt.float32, kind="ExternalInput").ap()
    if add_facet_bias:
        fbb = nc.dram_tensor("fbb", [128, NPAIR, 512], dt.float32, kind="ExternalInput").ap()
    # osc rows 32*(c%4)+b: cols 0..NCH-1 = per-chunk sum(exp(z)), col NCH+b = z_target
    osc = nc.dram_tensor("osc", [128, NCH + 2], dt.float32, kind="ExternalOutput").ap()
    ost = nc.dram_tensor("ost", [B, D], dt.float32, kind="ExternalOutput").ap()

    with tile.TileContext(nc) as tc:
        with (
            tc.tile_pool(name="const", bufs=1) as const,
            tc.tile_pool(name="fwp", bufs=NPAIR) as fwp,     # fully resident
            tc.tile_pool(name="swp", bufs=NSB) as swp,
            tc.tile_pool(name="scratch", bufs=2) as scratch,
            tc.tile_pool(name="fbp", bufs=2) as fbp,
            tc.tile_pool(name="psf", bufs=3, space="PSUM") as psf,
            tc.tile_pool(name="pss", bufs=2, space="PSUM") as pss,
        ):
            # tiny h loads ride the SWDGE (gpsimd) rings so the sync HWDGE
            # FIFO starts streaming weights immediately
            hb = const.tile([128, NKC * B], hdt)
            nc.gpsimd.dma_start(hb[:], hTb)
            hf = const.tile([128, NKC * B], dt.float32)
            nc.gpsimd.dma_start(hf[:], hTf)

            osc_sb = const.tile([128, NCH + 2], dt.float32)
            nc.gpsimd.memset(osc_sb[:], 0.0)
            ost_sb = const.tile([B, D], dt.float32)

            # ---- states first: the f32 (2-pass) matmuls and sw DMAs run
            # under the facet weight stream, keeping the kernel tail cheap.
            for sb_i in range(NSB):
                swt = swp.tile([128, NKC * 512], dt.float32)
                nc.sync.dma_start(swt[:], sw[:, sb_i, :])
                ss = pss.tile([B, 512], dt.float32)
                for kc in range(NKC):
                    nc.tensor.matmul(
                        ss[:],
                        hf[:, kc * B:(kc + 1) * B],
                        swt[:, kc * 512:(kc + 1) * 512],
                        start=(kc == 0),
                        stop=(kc == NKC - 1),
                    )
                nc.scalar.copy(ost_sb[:, sb_i * 512:(sb_i + 1) * 512], ss[:])
            nc.scalar.dma_start(ost, ost_sb[:])

            # ---- facet: 4 v-chunks packed per PSUM bank via PE column
            # tiling; 4 matmuls stream concurrently through distinct column
            # groups of the array.
            for t in range(NPAIR):
                fwt = fwp.tile([128, NKC * 4 * 512], fdt)
                nc.sync.dma_start(fwt[:], fw[:, t, :])
                pt = psf.tile([128, 512], dt.float32)
                for kc in range(NKC):
                    for cc in range(4):
                        nc.tensor.matmul(
                            pt[32 * cc:32 * cc + B, :],
                            hb[:, kc * B:(kc + 1) * B],
                            fwt[:, kc * 2048 + cc * 512: kc * 2048 + cc * 512 + 512],
                            start=(kc == 0),
                            stop=(kc == NKC - 1),
                            tile_position=(0, 32 * cc),
                        )
                if add_facet_bias:
                    fbt = fbp.tile([128, 512], dt.float32)
                    nc.sync.dma_start(fbt[:], fbb[:, t, :])
                    nc.vector.tensor_add(pt[:], pt[:], fbt[:])
                ex = scratch.tile([128, 512], dt.float32)
                for cc in range(4):
                    c = t * 4 + cc
                    nc.scalar.activation(
                        ex[32 * cc:32 * cc + B, :],
                        pt[32 * cc:32 * cc + B, :],
                        mybir.ActivationFunctionType.Exp,
                        accum_out=osc_sb[32 * cc:32 * cc + B, c: c + 1],
                    )
                for b in range(B):
                    if t_idx[b] // 2048 == t:
                        j = (t_idx[b] // 512) % 4
                        off = t_idx[b] % 512
                        nc.scalar.copy(
                            osc_sb[32 * j:32 * j + B, NCH + b: NCH + b + 1],
                            pt[32 * j:32 * j + B, off: off + 1],
                        )

            nc.scalar.dma_start(osc, osc_sb[:])

    nc.compile()
    return nc


def _prep_core_inputs(h_last, facet_w, facet_b, state_w, add_facet_bias):
    """Per-core input dicts (expert-parallel: core k owns expert k)."""
    hT = np.ascontiguousarray(h_last.T.astype(np.float32))          # [D, B]
    hpre = hT.reshape(NKC, 128, B).transpose(1, 0, 2).reshape(128, NKC * B)
    hTf = np.ascontiguousarray(hpre)
    hTb = np.ascontiguousarray(hpre.astype(H_FACET_NP))

    in_maps = []
    for k in range(K):
        A = facet_w[k]                                              # [D, V] f32
        fw_pre = np.ascontiguousarray(
            A.reshape(NKC, 128, NPAIR, 4, 512).transpose(1, 2, 0, 3, 4)
        ).astype(_FACET_NP).reshape(128, NPAIR, NKC * 4 * 512)
        S = state_w[k].astype(np.float32)                           # [D, D]
        sw_pre = np.ascontiguousarray(
            S.reshape(NKC, 128, NSB, 512).transpose(1, 2, 0, 3)
        ).reshape(128, NSB, NKC * 512)
        m = {"hTb": hTb, "hTf": hTf, "fw": fw_pre, "sw": sw_pre}
        if add_facet_bias:
            fbb = np.zeros((128, NPAIR, 512), np.float32)
            fb = facet_b[k].astype(np.float32)
            for c in range(NCH):
                pair, j = c // 4, c % 4
                fbb[32 * j:32 * j + B, pair, :] = fb[c * 512:(c + 1) * 512]
            m["fbb"] = fbb
        in_maps.append(m)
    return in_maps


def _run_device(t_idx, add_facet_bias, in_maps, trace=False):
    from concourse.bass_utils import run_bass_kernel_spmd

    key = (tuple(t_idx), bool(add_facet_bias))
    nc = _PROGRAM_CACHE.get(key)
    if nc is None:
        nc = _build_program(t_idx, add_facet_bias)
        _PROGRAM_CACHE[key] = nc
    res = run_bass_kernel_spmd(
        nc, in_maps, list(range(K)),
        trace=trace, trace_cores=list(range(K)) if trace else None,
    )
    return res


def kernel(h, targets, em_gate_w, em_gate_b, state_w, state_b,
           mfs_gate_w, mfs_gate_b, facet_w, facet_b,
           ln_scale, ln_bias, compress_w, compress_b,
           _trace=False, _result_box=None):
    h = np.asarray(h)
    targets = np.asarray(targets)

    h_last = h[:, -1, :].astype(np.float64)                          # [B, D]
    t_idx = [int(targets[b, -1]) for b in range(B)]
    add_facet_bias = bool(np.any(np.asarray(facet_b)))

    in_maps = _prep_core_inputs(h_last, np.asarray(facet_w, np.float32),
                                np.asarray(facet_b, np.float32),
                                np.asarray(state_w), add_facet_bias)
    res = _run_device(t_idx, add_facet_bias, in_maps, trace=_trace)
    if _result_box is not None:
        _result_box.append(res)

    # ---- host combine (tiny: [B, K] logic + LN + compress) ----
    sumexp = np.zeros((B, K))
    z_t = np.zeros((B, K))
    states = np.zeros((B, K, D))
    for k in range(K):
        osc = res.results[k]["osc"].astype(np.float64)               # [128, NCH+2]
        for c in range(NCH):
            j = c % 4
            for b in range(B):
                sumexp[b, k] += osc[32 * j + b, c]
        for b in range(B):
            jb = (t_idx[b] // 512) % 4
            z_t[b, k] = osc[32 * jb + b, NCH + b]
        states[:, k, :] = res.results[k]["ost"].astype(np.float64)
    states += np.asarray(state_b, np.float64)[None, :, :]
    logp = z_t - np.log(sumexp)                                      # [B, K]

    def softmax64(x):
        e = np.exp(x - x.max(-1, keepdims=True))
        return e / e.sum(-1, keepdims=True)

    G = softmax64(h_last @ np.asarray(em_gate_w, np.float64)
                  + np.asarray(em_gate_b, np.float64))
    g = softmax64(h_last @ np.asarray(mfs_gate_w, np.float64)
                  + np.asarray(mfs_gate_b, np.float64))

    seli2 = np.argsort(-G, axis=-1, kind="stable")[:, :2]            # top-2, ties->low idx
    sel_mask = np.zeros((B, K), bool)
    for b in range(B):
        sel_mask[b, seli2[b]] = True

    logg = np.log(np.maximum(g, 1e-9))
    mix = logg + logp
    mmax = mix.max(-1, keepdims=True)
    log_mix = mmax[..., 0] + np.log(np.exp(mix - mmax).sum(-1))
    s = logp - log_mix[..., None]

    aha = (s > S_THRESH) & (~sel_mask)
    boosted = G * np.where(aha, BOOST_GAIN, 1.0)
    sel_add = np.zeros((B, K))
    for b in range(B):
        sel_add[b, seli2[b, 0]] = 0.5
    boosted = np.where(aha.any(-1, keepdims=True), boosted + sel_add, boosted)
    boosted = boosted / np.maximum(boosted.sum(-1, keepdims=True), EPS)

    bvec = np.einsum("bk,bkd->bd", boosted, states)
    mu = bvec.mean(-1, keepdims=True)
    var = ((bvec - mu) ** 2).mean(-1, keepdims=True)
    ln = (bvec - mu) / np.sqrt(var + 1e-5) * np.asarray(ln_scale, np.float64) \
         + np.asarray(ln_bias, np.float64)
    out = ln @ np.asarray(compress_w, np.float64) + np.asarray(compress_b, np.float64)
    return out.astype(np.float32)


# revision 9
# speedup vs baseline: 1.6262x; 1.0635x over previous
"""AhaDiffuser Trainium2 kernel.

Key algebraic fact: the reference returns b[:, -1, :] and every op is
pointwise in t, so the output depends only on h[:, -1, :] ([B, D]) and
targets[:, -1] ([B]).  The remaining heavy work is streaming the facet
(K x D x V) and state (K x D x D) weights through the TensorEngine once,
which is HBM-bandwidth bound.

Sharding (expert-parallel, per the hint): core k owns facet_w[k]/facet_b[k]
and state_w[k].  Each core computes, for its expert:
  z      = h_last @ facet_w[k]            [B, V]   (bf16 weights; decisions
                                                    have |s-0.7| margin ~0.25,
                                                    bf16 error ~0.003)
  sumexp = sum_v exp(z)  (per B, fused exp+accum on ScalarE)
  z_t    = z[b, targets[b, -1]]           (static offsets baked at build)
  states = h_last @ state_w[k]            [B, D]   (f32)
Host gathers the tiny [B] partials + [B, D] states, then does the [B, K]
gate/boost logic, combine, layernorm and compress in float64.
"""

import numpy as np
import ml_dtypes

B, T, D, K, V = 2, 1024, 1024, 8, 8192
NKC = D // 128            # contraction chunks of 128
NVB = 8                   # facet V blocks per core
VB = V // NVB             # 1024 columns per block
NSB = 2                   # state output-D blocks (512 each)
NCH = 16                  # facet v-chunks of 512 (col-tiled 4-per-PSUM-bank)
# facet DMA groups (chunk_start, n_chunks): big groups stream first; the
# last groups are small so the post-DMA matmul/exp tail is short
FGROUPS = [(0, 4), (4, 4), (8, 4), (12, 2), (14, 2)]
def _fgroup_of(c):
    for c0, n in FGROUPS:
        if c0 <= c < c0 + n:
            return c0
    raise ValueError(c)
S_THRESH, BOOST_GAIN, MAX_PAIRS, EPS = 0.7, 2.0, 1, 1e-9

FACET_DT_NAME = "float8e4"          # facet weights dtype; decisions have
                                    # |s-0.7| margin ~0.25 vs ~0.05 fp8 error
_FACET_NP = {"bfloat16": ml_dtypes.bfloat16,
             "float8e4": ml_dtypes.float8_e4m3}[FACET_DT_NAME]
H_FACET_NP = ml_dtypes.bfloat16     # stationary h dtype for the facet matmul

_PROGRAM_CACHE = {}


def _build_program(t_idx, add_facet_bias):
    import concourse.bacc as bacc
    import concourse.tile as tile
    import concourse.mybir as mybir

    dt = mybir.dt
    fdt = getattr(dt, FACET_DT_NAME)
    hdt = dt.bfloat16

    nc = bacc.Bacc("TRN2", target_bir_lowering=False, debug=False)

    hTb = nc.dram_tensor("hTb", [128, NKC * B], hdt, kind="ExternalInput").ap()
    hTf = nc.dram_tensor("hTf", [128, NKC * B], dt.float32, kind="ExternalInput").ap()
    # facet weights chunk-major: [128, chunk, (kc, 512)] so any contiguous
    # chunk range is one contiguous-per-partition DMA
    fw = nc.dram_tensor("fw", [128, NCH, NKC * 512], fdt, kind="ExternalInput").ap()
    sw = nc.dram_tensor("sw", [128, NSB, NKC * 512], dt.float32, kind="ExternalInput").ap()
    if add_facet_bias:
        fbb = nc.dram_tensor("fbb", [128, NCH, 512], dt.float32, kind="ExternalInput").ap()
    # osc rows 32*(c%4)+b: cols 0..NCH-1 = per-chunk sum(exp(z)), col NCH+b = z_target
    osc = nc.dram_tensor("osc", [128, NCH + 2], dt.float32, kind="ExternalOutput").ap()
    ost = nc.dram_tensor("ost", [B, D], dt.float32, kind="ExternalOutput").ap()

    with tile.TileContext(nc) as tc:
        with (
            tc.tile_pool(name="const", bufs=1) as const,
            tc.tile_pool(name="fwp", bufs=len(FGROUPS)) as fwp,  # fully resident
            tc.tile_pool(name="swp", bufs=NSB) as swp,
            tc.tile_pool(name="scratch", bufs=2) as scratch,
            tc.tile_pool(name="fbp", bufs=2) as fbp,
            tc.tile_pool(name="psf", bufs=3, space="PSUM") as psf,
            tc.tile_pool(name="pss", bufs=2, space="PSUM") as pss,
        ):
            # tiny h loads ride the SWDGE (gpsimd) rings so the sync HWDGE
            # FIFO starts streaming weights immediately
            hb = const.tile([128, NKC * B], hdt)
            nc.gpsimd.dma_start(hb[:], hTb)
            hf = const.tile([128, NKC * B], dt.float32)
            nc.gpsimd.dma_start(hf[:], hTf)

            osc_sb = const.tile([128, NCH + 2], dt.float32)
            nc.gpsimd.memset(osc_sb[:], 0.0)
            ost_sb = const.tile([B, D], dt.float32)

            # ---- states first: the f32 (2-pass) matmuls and sw DMAs run
            # under the facet weight stream, keeping the kernel tail cheap.
            for sb_i in range(NSB):
                swt = swp.tile([128, NKC * 512], dt.float32)
                nc.sync.dma_start(swt[:], sw[:, sb_i, :])
                ss = pss.tile([B, 512], dt.float32)
                for kc in range(NKC):
                    nc.tensor.matmul(
                        ss[:],
                        hf[:, kc * B:(kc + 1) * B],
                        swt[:, kc * 512:(kc + 1) * 512],
                        start=(kc == 0),
                        stop=(kc == NKC - 1),
                    )
                nc.scalar.copy(ost_sb[:, sb_i * 512:(sb_i + 1) * 512], ss[:])
            nc.scalar.dma_start(ost, ost_sb[:])

            # ---- facet: up to 4 v-chunks packed per PSUM bank via PE column
            # tiling; the packed matmuls stream concurrently through distinct
            # column groups of the array.
            for c0, n in FGROUPS:
                fwt = fwp.tile([128, n * NKC * 512], fdt, tag=f"fw{n}")
                nc.sync.dma_start(fwt[:], fw[:, c0:c0 + n, :])
                pt = psf.tile([128, 512], dt.float32)
                for kc in range(NKC):
                    for i in range(n):
                        nc.tensor.matmul(
                            pt[32 * i:32 * i + B, :],
                            hb[:, kc * B:(kc + 1) * B],
                            fwt[:, i * NKC * 512 + kc * 512: i * NKC * 512 + (kc + 1) * 512],
                            start=(kc == 0),
                            stop=(kc == NKC - 1),
                            tile_position=(0, 32 * i),
                        )
                if add_facet_bias:
                    fbt = fbp.tile([128, NCH, 512], dt.float32, tag="fbt")
                    nc.sync.dma_start(fbt[:, :n, :], fbb[:, c0:c0 + n, :])
                    for i in range(n):
                        nc.vector.tensor_add(pt[32 * i:32 * i + B, :],
                                             pt[32 * i:32 * i + B, :],
                                             fbt[32 * i:32 * i + B, i, :])
                ex = scratch.tile([128, 512], dt.float32)
                for i in range(n):
                    c = c0 + i
                    nc.scalar.activation(
                        ex[32 * i:32 * i + B, :],
                        pt[32 * i:32 * i + B, :],
                        mybir.ActivationFunctionType.Exp,
                        accum_out=osc_sb[32 * i:32 * i + B, c: c + 1],
                    )
                for b in range(B):
                    if _fgroup_of(t_idx[b] // 512) == c0:
                        i = t_idx[b] // 512 - c0
                        off = t_idx[b] % 512
                        nc.scalar.copy(
                            osc_sb[32 * i:32 * i + B, NCH + b: NCH + b + 1],
                            pt[32 * i:32 * i + B, off: off + 1],
                        )

            nc.scalar.dma_start(osc, osc_sb[:])

    nc.compile()
    return nc


def _prep_core_inputs(h_last, facet_w, facet_b, state_w, add_facet_bias):
    """Per-core input dicts (expert-parallel: core k owns expert k)."""
    hT = np.ascontiguousarray(h_last.T.astype(np.float32))          # [D, B]
    hpre = hT.reshape(NKC, 128, B).transpose(1, 0, 2).reshape(128, NKC * B)
    hTf = np.ascontiguousarray(hpre)
    hTb = np.ascontiguousarray(hpre.astype(H_FACET_NP))

    in_maps = []
    for k in range(K):
        A = facet_w[k]                                              # [D, V] f32
        fw_pre = np.ascontiguousarray(
            A.reshape(NKC, 128, NCH, 512).transpose(1, 2, 0, 3)
        ).astype(_FACET_NP).reshape(128, NCH, NKC * 512)
        S = state_w[k].astype(np.float32)                           # [D, D]
        sw_pre = np.ascontiguousarray(
            S.reshape(NKC, 128, NSB, 512).transpose(1, 2, 0, 3)
        ).reshape(128, NSB, NKC * 512)
        m = {"hTb": hTb, "hTf": hTf, "fw": fw_pre, "sw": sw_pre}
        if add_facet_bias:
            fbb = np.zeros((128, NCH, 512), np.float32)
            fb = facet_b[k].astype(np.float32)
            for c in range(NCH):
                i = c - _fgroup_of(c)
                fbb[32 * i:32 * i + B, c, :] = fb[c * 512:(c + 1) * 512]
            m["fbb"] = fbb
        in_maps.append(m)
    return in_maps


def _run_device(t_idx, add_facet_bias, in_maps, trace=False):
    from concourse.bass_utils import run_bass_kernel_spmd

    key = (tuple(t_idx), bool(add_facet_bias))
    nc = _PROGRAM_CACHE.get(key)
    if nc is None:
        nc = _build_program(t_idx, add_facet_bias)
        _PROGRAM_CACHE[key] = nc
    res = run_bass_kernel_spmd(
        nc, in_maps, list(range(K)),
        trace=trace, trace_cores=list(range(K)) if trace else None,
    )
    return res


def kernel(h, targets, em_gate_w, em_gate_b, state_w, state_b,
           mfs_gate_w, mfs_gate_b, facet_w, facet_b,
           ln_scale, ln_bias, compress_w, compress_b,
           _trace=False, _result_box=None):
    h = np.asarray(h)
    targets = np.asarray(targets)

    h_last = h[:, -1, :].astype(np.float64)                          # [B, D]
    t_idx = [int(targets[b, -1]) for b in range(B)]
    add_facet_bias = bool(np.any(np.asarray(facet_b)))

    in_maps = _prep_core_inputs(h_last, np.asarray(facet_w, np.float32),
                                np.asarray(facet_b, np.float32),
                                np.asarray(state_w), add_facet_bias)
    res = _run_device(t_idx, add_facet_bias, in_maps, trace=_trace)
    if _result_box is not None:
        _result_box.append(res)

    # ---- host combine (tiny: [B, K] logic + LN + compress) ----
    sumexp = np.zeros((B, K))
    z_t = np.zeros((B, K))
    states = np.zeros((B, K, D))
    for k in range(K):
        osc = res.results[k]["osc"].astype(np.float64)               # [128, NCH+2]
        for c in range(NCH):
            j = c - _fgroup_of(c)
            for b in range(B):
                sumexp[b, k] += osc[32 * j + b, c]
        for b in range(B):
            jb = t_idx[b] // 512 - _fgroup_of(t_idx[b] // 512)
            z_t[b, k] = osc[32 * jb + b, NCH + b]
        states[:, k, :] = res.results[k]["ost"].astype(np.float64)
    states += np.asarray(state_b, np.float64)[None, :, :]
    logp = z_t - np.log(sumexp)                                      # [B, K]

    def softmax64(x):
        e = np.exp(x - x.max(-1, keepdims=True))
        return e / e.sum(-1, keepdims=True)

    G = softmax64(h_last @ np.asarray(em_gate_w, np.float64)
                  + np.asarray(em_gate_b, np.float64))
    g = softmax64(h_last @ np.asarray(mfs_gate_w, np.float64)
                  + np.asarray(mfs_gate_b, np.float64))

    seli2 = np.argsort(-G, axis=-1, kind="stable")[:, :2]            # top-2, ties->low idx
    sel_mask = np.zeros((B, K), bool)
    for b in range(B):
        sel_mask[b, seli2[b]] = True

    logg = np.log(np.maximum(g, 1e-9))
    mix = logg + logp
    mmax = mix.max(-1, keepdims=True)
    log_mix = mmax[..., 0] + np.log(np.exp(mix - mmax).sum(-1))
    s = logp - log_mix[..., None]

    aha = (s > S_THRESH) & (~sel_mask)
    boosted = G * np.where(aha, BOOST_GAIN, 1.0)
    sel_add = np.zeros((B, K))
    for b in range(B):
        sel_add[b, seli2[b, 0]] = 0.5
    boosted = np.where(aha.any(-1, keepdims=True), boosted + sel_add, boosted)
    boosted = boosted / np.maximum(boosted.sum(-1, keepdims=True), EPS)

    bvec = np.einsum("bk,bkd->bd", boosted, states)
    mu = bvec.mean(-1, keepdims=True)
    var = ((bvec - mu) ** 2).mean(-1, keepdims=True)
    ln = (bvec - mu) / np.sqrt(var + 1e-5) * np.asarray(ln_scale, np.float64) \
         + np.asarray(ln_bias, np.float64)
    out = ln @ np.asarray(compress_w, np.float64) + np.asarray(compress_b, np.float64)
    return out.astype(np.float32)
